# revision 8
# baseline (speedup 1.0000x reference)
"""Trainium2 Bass kernel for nn_AgentModel (dense_mlp, 8-core data parallel).

Strategy
--------
- Pure data parallelism: batch B=8192 sharded as 1024 rows per core; all
  weights replicated (host pre-packs them).
- Host prep (free): transpose features, transpose/cast weights to fp16,
  fold W_out into W_ah (eliminates the out-proj GEMM), fold the attention
  1/sqrt(HD) scale into Wq, and fold the "+1-shifted ELU" corrections into
  downstream biases.
- ELU trick: selu1(z) := elu(z)+1 = max(z+1, exp(min(z,0))). All ELU outputs
  feed only linear layers, so the +1 shift is corrected by subtracting
  W.sum(in_axis) from the consumer's bias (done on host).
- Layouts: big GEMMs run "transposed" ([feature-dim on partitions, batch in
  free dim], weights stationary). q/k/v are produced in natural layout
  ([batch on partitions]) by using the transposed latents as the stationary
  operand. Attention (O=3 opponents, H=4 heads) is elementwise in natural
  layout; the attention output is PE-transposed back.
"""

import os
from contextlib import ExitStack

import numpy as np

import concourse.bass as bass
import concourse.mybir as mybir
from concourse import bacc
import concourse.tile as tile

F16 = mybir.dt.float16
F32 = mybir.dt.float32
AO = mybir.AluOpType
AF = mybir.ActivationFunctionType

B, F, U, O, A, H = 8192, 1024, 512, 3, 6, 4
HD = U // H          # 128 (head dim == one partition chunk)
NCORES = 8
BS = B // NCORES     # 1024 rows per core
NB = 512             # batch mega-tile width (one PSUM bank at fp32)
NMEGA = BS // NB     # 2
KF = F // 128        # 8 contraction chunks for F
KU = U // 128        # 4 contraction chunks for U
MU = U // 128        # 4 output chunks for U
PV = A + 1           # 7 used policy+value rows
PVP = 32             # padded matmul M (M<32 outputs crash the exec unit)


def _selu1_epilogue(nc, pools, ps, nb_ap, b1_ap, out_ap):
    """out = elu(psum + b) + 1 = max(psum + b + 1, exp(min(psum + b, 0))).

    nb_ap: per-partition [-b]  (fp32 [128,1])
    b1_ap: per-partition [b+1] (fp32 [128,1])
    """
    tneg = pools["tneg"].tile([128, NB], F16, tag="tneg")
    # tneg = relu(-(psum + b)) = -min(z, 0)
    nc.scalar.activation(tneg[:], ps[:], AF.Relu, bias=nb_ap, scale=-1.0)
    e = pools["e"].tile([128, NB], F16, tag="e")
    # e = exp(-tneg) = exp(min(z, 0))
    nc.scalar.activation(e[:], tneg[:], AF.Exp, scale=-1.0)
    # out = (psum + (b+1)) max e
    nc.vector.scalar_tensor_tensor(
        out=out_ap, in0=ps[:], scalar=b1_ap, in1=e[:], op0=AO.add, op1=AO.max
    )


def build_nc():
    nc = bacc.Bacc("TRN2", target_bir_lowering=False, debug=False, num_devices=NCORES)

    # ---- DRAM I/O ----
    xT = nc.dram_tensor("xT", [F, BS], F16, kind="ExternalInput")
    w_al = nc.dram_tensor("w_al_T", [F, U], F16, kind="ExternalInput")
    w_ol = nc.dram_tensor("w_ol_T", [O, F, U], F16, kind="ExternalInput")
    w_q = nc.dram_tensor("w_q_T", [U, U], F16, kind="ExternalInput")
    w_k = nc.dram_tensor("w_k_T", [U, U], F16, kind="ExternalInput")
    w_v = nc.dram_tensor("w_v_T", [U, U], F16, kind="ExternalInput")
    bq_r = nc.dram_tensor("bq_row", [1, U], F16, kind="ExternalInput")
    bk_r = nc.dram_tensor("bk_row", [1, U], F16, kind="ExternalInput")
    bv_r = nc.dram_tensor("bv_row", [1, U], F16, kind="ExternalInput")
    w_ah = nc.dram_tensor("w_ah_T", [2 * U, U], F16, kind="ExternalInput")
    w_oh = nc.dram_tensor("w_oh_T", [O, U, U], F16, kind="ExternalInput")
    w_pva = nc.dram_tensor("w_pv_ag_T", [U, PVP], F16, kind="ExternalInput")
    w_pvo = nc.dram_tensor("w_pv_op_T", [O, U, PVP], F16, kind="ExternalInput")
    nb_al = nc.dram_tensor("nb_al", [128, MU], F32, kind="ExternalInput")
    b1_al = nc.dram_tensor("b1_al", [128, MU], F32, kind="ExternalInput")
    nb_ol = nc.dram_tensor("nb_ol", [O, 128, MU], F32, kind="ExternalInput")
    b1_ol = nc.dram_tensor("b1_ol", [O, 128, MU], F32, kind="ExternalInput")
    nb_ah = nc.dram_tensor("nb_ah", [128, MU], F32, kind="ExternalInput")
    b1_ah = nc.dram_tensor("b1_ah", [128, MU], F32, kind="ExternalInput")
    nb_oh = nc.dram_tensor("nb_oh", [O, 128, MU], F32, kind="ExternalInput")
    b1_oh = nc.dram_tensor("b1_oh", [O, 128, MU], F32, kind="ExternalInput")
    b_pv = nc.dram_tensor("b_pv", [PVP, 1 + O], F32, kind="ExternalInput")
    ident = nc.dram_tensor("ident", [128, 128], F16, kind="ExternalInput")

    o_agpv = nc.dram_tensor("out_ag_pv", [PV, BS], F32, kind="ExternalOutput")
    o_oppv = nc.dram_tensor("out_op_pv", [O, PV, BS], F32, kind="ExternalOutput")
    o_infl = nc.dram_tensor("out_infl", [BS, O], F32, kind="ExternalOutput")

    with tile.TileContext(nc) as tc, ExitStack() as ctx:
        wp = ctx.enter_context(tc.tile_pool(name="weights", bufs=1))
        fp = ctx.enter_context(tc.tile_pool(name="feat", bufs=2))
        actp = ctx.enter_context(tc.tile_pool(name="acts", bufs=8))
        opp = ctx.enter_context(tc.tile_pool(name="oacts", bufs=14))
        qkvp = ctx.enter_context(tc.tile_pool(name="qkv", bufs=2))
        scr = ctx.enter_context(tc.tile_pool(name="scr", bufs=4))
        smp = ctx.enter_context(tc.tile_pool(name="small", bufs=4))
        psp = ctx.enter_context(tc.tile_pool(name="psum", bufs=4, space="PSUM"))
        pools = {"tneg": scr, "e": scr}

        dma = nc.sync.dma_start

        # ---- load weights into SBUF (once) ----
        def load_w(dram_ap, kchunks, name):
            t = wp.tile(
                [128, kchunks, dram_ap.shape[-1]], F16, tag=name, name=name
            )
            dma(out=t[:], in_=dram_ap.rearrange("(k p) u -> p k u", p=128))
            return t

        w_al_sb = load_w(w_al[:], KF, "w_al_sb")
        w_ol_sb = [load_w(w_ol[o], KF, f"w_ol_sb{o}") for o in range(O)]
        w_q_sb = load_w(w_q[:], KU, "w_q_sb")
        w_k_sb = load_w(w_k[:], KU, "w_k_sb")
        w_v_sb = load_w(w_v[:], KU, "w_v_sb")
        w_ah_sb = load_w(w_ah[:], 2 * KU, "w_ah_sb")
        w_oh_sb = [load_w(w_oh[o], KU, f"w_oh_sb{o}") for o in range(O)]
        w_pva_sb = load_w(w_pva[:], KU, "w_pva_sb")
        w_pvo_sb = [load_w(w_pvo[o], KU, f"w_pvo_sb{o}") for o in range(O)]

        def load_small(dram_ap, shape, dt, name):
            t = wp.tile(shape, dt, tag=name, name=name)
            dma(out=t[:], in_=dram_ap)
            return t

        bq_sb = load_small(bq_r[:], [1, U], F16, "bq_sb")
        bk_sb = load_small(bk_r[:], [1, U], F16, "bk_sb")
        bv_sb = load_small(bv_r[:], [1, U], F16, "bv_sb")
        nb_al_sb = load_small(nb_al[:], [128, MU], F32, "nb_al_sb")
        b1_al_sb = load_small(b1_al[:], [128, MU], F32, "b1_al_sb")
        nb_ol_sb = [
            load_small(nb_ol[o], [128, MU], F32, f"nb_ol_sb{o}") for o in range(O)
        ]
        b1_ol_sb = [
            load_small(b1_ol[o], [128, MU], F32, f"b1_ol_sb{o}") for o in range(O)
        ]
        nb_ah_sb = load_small(nb_ah[:], [128, MU], F32, "nb_ah_sb")
        b1_ah_sb = load_small(b1_ah[:], [128, MU], F32, "b1_ah_sb")
        nb_oh_sb = [
            load_small(nb_oh[o], [128, MU], F32, f"nb_oh_sb{o}") for o in range(O)
        ]
        b1_oh_sb = [
            load_small(b1_oh[o], [128, MU], F32, f"b1_oh_sb{o}") for o in range(O)
        ]
        b_pv_sb = load_small(b_pv[:], [PVP, 1 + O], F32, "b_pv_sb")
        ident_sb = load_small(ident[:], [128, 128], F16, "ident_sb")
        ones_sb = wp.tile([1, 128], F16)
        nc.vector.memset(ones_sb[:], 1.0)

        for m in range(NMEGA):
            bcol = m * NB

            # ---------- stage A: latents (transposed out, f-contraction) ----
            feat = fp.tile([128, KF, NB], F16, tag="feat")
            dma(
                out=feat[:],
                in_=xT.rearrange("(k p) b -> p k b", p=128)[
                    :, :, bcol : bcol + NB
                ],
            )

            lat1 = []  # 4 tiles [128, NB] fp16, chunk mc = u-dims [128mc, 128mc+128)
            for mc in range(MU):
                ps = psp.tile([128, NB], F32, tag="mm")
                for k in range(KF):
                    nc.tensor.matmul(
                        ps[:],
                        w_al_sb[:, k, 128 * mc : 128 * (mc + 1)],
                        feat[:, k, :],
                        start=(k == 0),
                        stop=(k == KF - 1),
                    )
                out = actp.tile([128, NB], F16, tag="lat1")
                _selu1_epilogue(
                    nc, pools, ps,
                    nb_al_sb[:, mc : mc + 1], b1_al_sb[:, mc : mc + 1], out[:],
                )
                lat1.append(out)

            opl1 = []  # [o][mc]
            for o in range(O):
                tiles = []
                for mc in range(MU):
                    ps = psp.tile([128, NB], F32, tag="mm")
                    for k in range(KF):
                        nc.tensor.matmul(
                            ps[:],
                            w_ol_sb[o][:, k, 128 * mc : 128 * (mc + 1)],
                            feat[:, k, :],
                            start=(k == 0),
                            stop=(k == KF - 1),
                        )
                    out = opp.tile([128, NB], F16, tag="opl1")
                    _selu1_epilogue(
                        nc, pools, ps,
                        nb_ol_sb[o][:, mc : mc + 1],
                        b1_ol_sb[o][:, mc : mc + 1],
                        out[:],
                    )
                    tiles.append(out)
                opl1.append(tiles)

            # ---------- stage B: q/k/v (natural layout) + attention ----------
            attn_T = [
                actp.tile([128, NB], F16, tag="attnT", name=f"attn_T{m}_{uc}")
                for uc in range(MU)
            ]
            for j in range(NB // 128):  # b-chunks of 128 within the mega-tile
                jcol = 128 * j

                def qkv_gemm(lat_tiles, w_sb, brow_sb, tag):
                    ps = psp.tile([128, NB], F32, tag="mm")
                    for k in range(KU):
                        nc.tensor.matmul(
                            ps[:],
                            lat_tiles[k][:, jcol : jcol + 128],
                            w_sb[:, k, :],
                            start=(k == 0),
                            stop=False,
                        )
                    # bias along the free dim via a K=1 ones matmul
                    nc.tensor.matmul(
                        ps[:], ones_sb[:], brow_sb[:], start=False, stop=True
                    )
                    sb = qkvp.tile([128, NB], F16, tag=tag)
                    nc.scalar.copy(sb[:], ps[:])
                    return sb

                q_sb = qkv_gemm(lat1, w_q_sb, bq_sb, "q")
                k_sbs = [qkv_gemm(opl1[o], w_k_sb, bk_sb, f"k{o}") for o in range(O)]
                v_sbs = [qkv_gemm(opl1[o], w_v_sb, bv_sb, f"v{o}") for o in range(O)]

                # scores[b,(o,h)] = sum_d q*k  -> [128, O*H] fp32
                sc = smp.tile([128, O * H], F32, tag="sc")
                for o in range(O):
                    prod = scr.tile([128, NB], F16, tag="prod", bufs=2)
                    nc.vector.tensor_mul(prod[:], q_sb[:], k_sbs[o][:])
                    nc.vector.tensor_reduce(
                        out=sc[:, H * o : H * (o + 1)],
                        in_=prod[:].rearrange("p (h d) -> p h d", h=H),
                        axis=mybir.AxisListType.X,
                        op=AO.add,
                    )
                # softmax over o (scores are tiny; skip the max-subtraction)
                e12 = smp.tile([128, O * H], F32, tag="e12")
                nc.scalar.activation(e12[:], sc[:], AF.Exp)
                s4 = smp.tile([128, H], F32, tag="s4")
                nc.vector.tensor_add(s4[:], e12[:, 0:H], e12[:, H : 2 * H])
                nc.vector.tensor_add(s4[:], s4[:], e12[:, 2 * H : 3 * H])
                r4 = smp.tile([128, H], F32, tag="r4")
                nc.vector.reciprocal(r4[:], s4[:])

                # opponent influences: infl[b,o] = mean_h w[b,h,o]
                infl = smp.tile([128, O], F32, tag="infl")
                wn = smp.tile([128, O * H], F32, tag="wn")
                for o in range(O):
                    nc.vector.scalar_tensor_tensor(
                        out=wn[:, H * o : H * (o + 1)],
                        in0=e12[:, H * o : H * (o + 1)],
                        scalar=1.0 / H,
                        in1=r4[:],
                        op0=AO.mult,
                        op1=AO.mult,
                    )
                nc.vector.tensor_reduce(
                    out=infl[:],
                    in_=wn[:].rearrange("p (o h) -> p o h", o=O),
                    axis=mybir.AxisListType.X,
                    op=AO.add,
                )
                dma(out=o_infl[bcol + jcol : bcol + jcol + 128, :], in_=infl[:])

                # attn (unnormalized): P2[:, o, (h d)] = e[b,(o,h)] * v_o[b,(h,d)]
                p2 = scr.tile([128, O * NB], F16, tag="p2", bufs=2)
                for o in range(O):
                    for h in range(H):
                        nc.vector.tensor_scalar(
                            out=p2[:, o * NB + HD * h : o * NB + HD * (h + 1)],
                            in0=v_sbs[o][:, HD * h : HD * (h + 1)],
                            scalar1=e12[:, H * o + h : H * o + h + 1],
                            scalar2=None,
                            op0=AO.mult,
                        )
                att_u = scr.tile([128, NB], F16, tag="att_u", bufs=2)
                nc.vector.tensor_add(att_u[:], p2[:, 0:NB], p2[:, NB : 2 * NB])
                nc.vector.tensor_add(att_u[:], att_u[:], p2[:, 2 * NB : 3 * NB])
                att_n = scr.tile([128, NB], F16, tag="att_n", bufs=2)
                for h in range(H):
                    nc.vector.tensor_scalar(
                        out=att_n[:, HD * h : HD * (h + 1)],
                        in0=att_u[:, HD * h : HD * (h + 1)],
                        scalar1=r4[:, h : h + 1],
                        scalar2=None,
                        op0=AO.mult,
                    )
                # transpose attn back to [u, b] chunks
                for uc in range(MU):
                    tp = psp.tile([128, 128], F16, tag="tp")
                    nc.tensor.transpose(
                        tp[:], att_n[:, 128 * uc : 128 * (uc + 1)], ident_sb[:]
                    )
                    nc.scalar.copy(attn_T[uc][:, jcol : jcol + 128], tp[:])

            # ---------- stage D: heads (transposed out) ----------
            head1 = []
            for mc in range(MU):
                ps = psp.tile([128, NB], F32, tag="mm")
                for k in range(2 * KU):
                    rhs = lat1[k][:] if k < KU else attn_T[k - KU][:]
                    nc.tensor.matmul(
                        ps[:],
                        w_ah_sb[:, k, 128 * mc : 128 * (mc + 1)],
                        rhs,
                        start=(k == 0),
                        stop=(k == 2 * KU - 1),
                    )
                out = actp.tile([128, NB], F16, tag="head1", bufs=4)
                _selu1_epilogue(
                    nc, pools, ps,
                    nb_ah_sb[:, mc : mc + 1], b1_ah_sb[:, mc : mc + 1], out[:],
                )
                head1.append(out)

            oh1 = []
            for o in range(O):
                tiles = []
                for mc in range(MU):
                    ps = psp.tile([128, NB], F32, tag="mm")
                    for k in range(KU):
                        nc.tensor.matmul(
                            ps[:],
                            w_oh_sb[o][:, k, 128 * mc : 128 * (mc + 1)],
                            opl1[o][k][:],
                            start=(k == 0),
                            stop=(k == KU - 1),
                        )
                    out = opp.tile([128, NB], F16, tag="oh1")
                    _selu1_epilogue(
                        nc, pools, ps,
                        nb_oh_sb[o][:, mc : mc + 1],
                        b1_oh_sb[o][:, mc : mc + 1],
                        out[:],
                    )
                    tiles.append(out)
                oh1.append(tiles)

            # ---------- stage E: policies + values (packed PV=7 rows) -------
            def pv_gemm(w_sb, head_tiles, bias_col, out_dram_slice):
                ps = psp.tile([PVP, NB], F32, tag="mm")
                for k in range(KU):
                    nc.tensor.matmul(
                        ps[:],
                        w_sb[:, k, :],
                        head_tiles[k][:],
                        start=(k == 0),
                        stop=(k == KU - 1),
                    )
                sb = smp.tile([PVP, NB], F32, tag="pv", bufs=2)
                nc.scalar.activation(
                    sb[:], ps[:], AF.Identity,
                    bias=b_pv_sb[:, bias_col : bias_col + 1], scale=1.0,
                )
                dma(out=out_dram_slice, in_=sb[0:PV, :])

            pv_gemm(w_pva_sb, head1, 0, o_agpv[:, bcol : bcol + NB])
            for o in range(O):
                pv_gemm(
                    w_pvo_sb[o], oh1[o], 1 + o, o_oppv[o][:, bcol : bcol + NB]
                )

    nc.finalize()
    return nc


def pack_inputs(features, W_al, b_al, W_in, b_in, W_out, b_out, W_ah, b_ah,
                W_ap, b_ap, W_av, b_av, W_ol, b_ol, W_oh, b_oh, W_op, b_op,
                W_ov, b_ov):
    """Host-side packing: transpose/cast weights, fold scale/shift corrections."""
    f32 = np.float32
    f16 = np.float16
    np32 = lambda a: np.asarray(a, dtype=f32)

    features = np32(features)
    W_al, b_al = np32(W_al), np32(b_al)
    W_in, b_in = np32(W_in), np32(b_in)
    W_out, b_out = np32(W_out), np32(b_out)
    W_ah, b_ah = np32(W_ah), np32(b_ah)
    W_ap, b_ap = np32(W_ap), np32(b_ap)
    W_av, b_av = np32(W_av), np32(b_av)
    W_ol, b_ol = np32(W_ol), np32(b_ol)
    W_oh, b_oh = np32(W_oh), np32(b_oh)
    W_op, b_op = np32(W_op), np32(b_op)
    W_ov, b_ov = np32(W_ov), np32(b_ov)

    def bias_pair(b_eff):
        # per-partition bias tiles [128, MU] for the transposed-layout epilogue
        nb = (-b_eff).reshape(MU, 128).T.copy()
        b1 = (b_eff + 1.0).reshape(MU, 128).T.copy()
        return nb.astype(f32), b1.astype(f32)

    sc = 1.0 / np.sqrt(HD)
    Wq, Wk, Wv = W_in[:U] * sc, W_in[U : 2 * U], W_in[2 * U :]
    bq, bk, bv = b_in[:U] * sc, b_in[U : 2 * U], b_in[2 * U :]
    # +1-shift corrections: consumers of shifted activations subtract W @ 1
    bq_eff = bq - Wq.sum(1)
    bk_eff = bk - Wk.sum(1)
    bv_eff = bv - Wv.sum(1)

    W_ah1 = W_ah[:, :U]
    W_ah2 = W_ah[:, U:] @ W_out  # fold out-proj into the agent-head GEMM
    b_ah_eff = b_ah + W_ah[:, U:] @ b_out - W_ah1.sum(1)
    b_oh_eff = b_oh - W_oh.sum(2)  # [O, U]

    nb_al, b1_al = bias_pair(b_al)
    nb_ah, b1_ah = bias_pair(b_ah_eff)
    nb_ol = np.stack([bias_pair(b_ol[o])[0] for o in range(O)])
    b1_ol = np.stack([bias_pair(b_ol[o])[1] for o in range(O)])
    nb_oh = np.stack([bias_pair(b_oh_eff[o])[0] for o in range(O)])
    b1_oh = np.stack([bias_pair(b_oh_eff[o])[1] for o in range(O)])

    w_pv_ag = np.zeros((PVP, U), f32)
    w_pv_ag[:PV] = np.concatenate([W_ap, W_av], axis=0)     # [7, U] used
    b_pv_ag = np.zeros((PVP,), f32)
    b_pv_ag[:PV] = np.concatenate([b_ap - W_ap.sum(1), b_av - W_av.sum(1)])
    w_pv_op = np.zeros((O, PVP, U), f32)
    w_pv_op[:, :PV] = np.concatenate([W_op, W_ov], axis=1)  # [O, 7, U] used
    b_pv_op = np.zeros((O, PVP), f32)
    b_pv_op[:, :PV] = np.concatenate(
        [b_op - W_op.sum(2), b_ov - W_ov.sum(2)], axis=1
    )
    b_pv = np.concatenate([b_pv_ag[:, None], b_pv_op.transpose(1, 0)], axis=1).astype(f32)

    shared = {
        "w_al_T": np.ascontiguousarray(W_al.T, dtype=f16),
        "w_ol_T": np.ascontiguousarray(W_ol.transpose(0, 2, 1), dtype=f16),
        "w_q_T": np.ascontiguousarray(Wq.T, dtype=f16),
        "w_k_T": np.ascontiguousarray(Wk.T, dtype=f16),
        "w_v_T": np.ascontiguousarray(Wv.T, dtype=f16),
        "bq_row": bq_eff.reshape(1, U).astype(f16),
        "bk_row": bk_eff.reshape(1, U).astype(f16),
        "bv_row": bv_eff.reshape(1, U).astype(f16),
        "w_ah_T": np.ascontiguousarray(
            np.concatenate([W_ah1.T, W_ah2.T], axis=0), dtype=f16
        ),
        "w_oh_T": np.ascontiguousarray(W_oh.transpose(0, 2, 1), dtype=f16),
        "w_pv_ag_T": np.ascontiguousarray(w_pv_ag.T, dtype=f16),
        "w_pv_op_T": np.ascontiguousarray(w_pv_op.transpose(0, 2, 1), dtype=f16),
        "nb_al": nb_al, "b1_al": b1_al,
        "nb_ol": nb_ol, "b1_ol": b1_ol,
        "nb_ah": nb_ah, "b1_ah": b1_ah,
        "nb_oh": nb_oh, "b1_oh": b1_oh,
        "b_pv": b_pv,
        "ident": np.eye(128, dtype=f16),
    }

    xT_full = np.ascontiguousarray(features.T, dtype=f16)  # [F, B]
    in_maps = []
    for c in range(NCORES):
        m = dict(shared)
        m["xT"] = np.ascontiguousarray(xT_full[:, c * BS : (c + 1) * BS])
        in_maps.append(m)
    return in_maps


def assemble_outputs(results):
    """results: per-core dicts -> full-size output tuple (all fp32)."""
    agp, agv, opp_, opv, infl = [], [], [], [], []
    for r in results:
        ag = r["out_ag_pv"]            # [7, BS]
        op = r["out_op_pv"]            # [O, 7, BS]
        agp.append(ag[:A].T)           # [BS, A]
        agv.append(ag[A : A + 1].T)    # [BS, 1]
        opp_.append(op[:, :A].transpose(2, 0, 1))      # [BS, O, A]
        opv.append(op[:, A : A + 1].transpose(2, 0, 1))  # [BS, O, 1]
        infl.append(r["out_infl"])     # [BS, O]
    cat = lambda xs: np.ascontiguousarray(np.concatenate(xs, axis=0), np.float32)
    return (cat(agp), cat(agv), cat(opp_), cat(opv), cat(infl))


_NC_CACHE = None


def get_nc():
    global _NC_CACHE
    if _NC_CACHE is None:
        _NC_CACHE = build_nc()
    return _NC_CACHE


def kernel(**inputs):
    from concourse.bass_utils import run_bass_kernel_spmd

    nc = get_nc()
    in_maps = pack_inputs(**inputs)
    res = run_bass_kernel_spmd(nc, in_maps, list(range(NCORES)))
    return assemble_outputs(res.results)


# revision 9
# speedup vs baseline: 42.2727x; 42.2727x over previous
"""Trainium2 Bass kernel for nn_AgentModel (dense_mlp, 8-core data parallel).

Strategy
--------
- Pure data parallelism: batch B=8192 sharded as 1024 rows per core; all
  weights replicated (host pre-packs them).
- Host prep (free): transpose features, transpose/cast weights to fp16,
  fold W_out into W_ah (eliminates the out-proj GEMM), fold the attention
  1/sqrt(HD) scale into Wq, and fold the "+1-shifted ELU" corrections into
  downstream biases.
- ELU trick: selu1(z) := elu(z)+1 = max(z+1, exp(min(z,0))). All ELU outputs
  feed only linear layers, so the +1 shift is corrected by subtracting
  W.sum(in_axis) from the consumer's bias (done on host).
- Layouts: big GEMMs run "transposed" ([feature-dim on partitions, batch in
  free dim], weights stationary). q/k/v are produced in natural layout
  ([batch on partitions]) by using the transposed latents as the stationary
  operand. Attention (O=3 opponents, H=4 heads) is elementwise in natural
  layout; the attention output is PE-transposed back.
"""

import os
from contextlib import ExitStack

import numpy as np

import concourse.bass as bass
import concourse.mybir as mybir
from concourse import bacc
import concourse.tile as tile

F16 = mybir.dt.float16
F32 = mybir.dt.float32
AO = mybir.AluOpType
AF = mybir.ActivationFunctionType

B, F, U, O, A, H = 8192, 1024, 512, 3, 6, 4
HD = U // H          # 128 (head dim == one partition chunk)
NCORES = 8
BS = B // NCORES     # 1024 rows per core
NB = 512             # batch mega-tile width (one PSUM bank at fp32)
NMEGA = BS // NB     # 2
KF = F // 128        # 8 contraction chunks for F
KU = U // 128        # 4 contraction chunks for U
MU = U // 128        # 4 output chunks for U
PV = A + 1           # 7 used policy+value rows
PVP = 32             # padded matmul M (M<32 outputs crash the exec unit)


def _selu1_epilogue(nc, pools, ps, nb_ap, b1_ap, out_ap):
    """out = elu(psum + b) + 1 = max(psum + b + 1, exp(min(psum + b, 0))).

    nb_ap: per-partition [-b]  (fp32 [128,1])
    b1_ap: per-partition [b+1] (fp32 [128,1])
    """
    tneg = pools["tneg"].tile([128, NB], F16, tag="tneg")
    # tneg = relu(-(psum + b)) = -min(z, 0)
    nc.scalar.activation(tneg[:], ps[:], AF.Relu, bias=nb_ap, scale=-1.0)
    e = pools["e"].tile([128, NB], F16, tag="e")
    # e = exp(-tneg) = exp(min(z, 0))
    nc.scalar.activation(e[:], tneg[:], AF.Exp, scale=-1.0)
    # out = (psum + (b+1)) max e
    nc.vector.scalar_tensor_tensor(
        out=out_ap, in0=ps[:], scalar=b1_ap, in1=e[:], op0=AO.add, op1=AO.max
    )


def build_nc(niter=1):
    nc = bacc.Bacc("TRN2", target_bir_lowering=False, debug=False, num_devices=NCORES)

    # ---- DRAM I/O ----
    xT = nc.dram_tensor("xT", [F, BS], F16, kind="ExternalInput")
    w_al = nc.dram_tensor("w_al_T", [F, U], F16, kind="ExternalInput")
    w_ol = nc.dram_tensor("w_ol_T", [O, F, U], F16, kind="ExternalInput")
    w_q = nc.dram_tensor("w_q_T", [U, U], F16, kind="ExternalInput")
    w_k = nc.dram_tensor("w_k_T", [U, U], F16, kind="ExternalInput")
    w_v = nc.dram_tensor("w_v_T", [U, U], F16, kind="ExternalInput")
    bq_r = nc.dram_tensor("bq_row", [1, U], F16, kind="ExternalInput")
    bk_r = nc.dram_tensor("bk_row", [1, U], F16, kind="ExternalInput")
    bv_r = nc.dram_tensor("bv_row", [1, U], F16, kind="ExternalInput")
    w_ah = nc.dram_tensor("w_ah_T", [2 * U, U], F16, kind="ExternalInput")
    w_oh = nc.dram_tensor("w_oh_T", [O, U, U], F16, kind="ExternalInput")
    w_pva = nc.dram_tensor("w_pv_ag_T", [U, PVP], F16, kind="ExternalInput")
    w_pvo = nc.dram_tensor("w_pv_op_T", [O, U, PVP], F16, kind="ExternalInput")
    nb_al = nc.dram_tensor("nb_al", [128, MU], F32, kind="ExternalInput")
    b1_al = nc.dram_tensor("b1_al", [128, MU], F32, kind="ExternalInput")
    nb_ol = nc.dram_tensor("nb_ol", [O, 128, MU], F32, kind="ExternalInput")
    b1_ol = nc.dram_tensor("b1_ol", [O, 128, MU], F32, kind="ExternalInput")
    nb_ah = nc.dram_tensor("nb_ah", [128, MU], F32, kind="ExternalInput")
    b1_ah = nc.dram_tensor("b1_ah", [128, MU], F32, kind="ExternalInput")
    nb_oh = nc.dram_tensor("nb_oh", [O, 128, MU], F32, kind="ExternalInput")
    b1_oh = nc.dram_tensor("b1_oh", [O, 128, MU], F32, kind="ExternalInput")
    b_pv = nc.dram_tensor("b_pv", [PVP, 1 + O], F32, kind="ExternalInput")
    ident = nc.dram_tensor("ident", [128, 128], F16, kind="ExternalInput")

    o_agpv = nc.dram_tensor("out_ag_pv", [PV, BS], F32, kind="ExternalOutput")
    o_oppv = nc.dram_tensor("out_op_pv", [O, PV, BS], F32, kind="ExternalOutput")
    o_infl = nc.dram_tensor("out_infl", [BS, O], F32, kind="ExternalOutput")

    with tile.TileContext(nc) as tc, ExitStack() as ctx:
        wp = ctx.enter_context(tc.tile_pool(name="weights", bufs=1))
        fp = ctx.enter_context(tc.tile_pool(name="feat", bufs=2))
        actp = ctx.enter_context(tc.tile_pool(name="acts", bufs=8))
        opp = ctx.enter_context(tc.tile_pool(name="oacts", bufs=14))
        qkvp = ctx.enter_context(tc.tile_pool(name="qkv", bufs=2))
        scr = ctx.enter_context(tc.tile_pool(name="scr", bufs=4))
        smp = ctx.enter_context(tc.tile_pool(name="small", bufs=4))
        psp = ctx.enter_context(tc.tile_pool(name="psum", bufs=4, space="PSUM"))
        pools = {"tneg": scr, "e": scr}

        dma = nc.sync.dma_start

        # ---- load weights into SBUF (once) ----
        def load_w(dram_ap, kchunks, name):
            t = wp.tile(
                [128, kchunks, dram_ap.shape[-1]], F16, tag=name, name=name
            )
            dma(out=t[:], in_=dram_ap.rearrange("(k p) u -> p k u", p=128))
            return t

        w_al_sb = load_w(w_al[:], KF, "w_al_sb")
        w_ol_sb = [load_w(w_ol[o], KF, f"w_ol_sb{o}") for o in range(O)]
        w_q_sb = load_w(w_q[:], KU, "w_q_sb")
        w_k_sb = load_w(w_k[:], KU, "w_k_sb")
        w_v_sb = load_w(w_v[:], KU, "w_v_sb")
        w_ah_sb = load_w(w_ah[:], 2 * KU, "w_ah_sb")
        w_oh_sb = [load_w(w_oh[o], KU, f"w_oh_sb{o}") for o in range(O)]
        w_pva_sb = load_w(w_pva[:], KU, "w_pva_sb")
        w_pvo_sb = [load_w(w_pvo[o], KU, f"w_pvo_sb{o}") for o in range(O)]

        def load_small(dram_ap, shape, dt, name):
            t = wp.tile(shape, dt, tag=name, name=name)
            dma(out=t[:], in_=dram_ap)
            return t

        bq_sb = load_small(bq_r[:], [1, U], F16, "bq_sb")
        bk_sb = load_small(bk_r[:], [1, U], F16, "bk_sb")
        bv_sb = load_small(bv_r[:], [1, U], F16, "bv_sb")
        nb_al_sb = load_small(nb_al[:], [128, MU], F32, "nb_al_sb")
        b1_al_sb = load_small(b1_al[:], [128, MU], F32, "b1_al_sb")
        nb_ol_sb = [
            load_small(nb_ol[o], [128, MU], F32, f"nb_ol_sb{o}") for o in range(O)
        ]
        b1_ol_sb = [
            load_small(b1_ol[o], [128, MU], F32, f"b1_ol_sb{o}") for o in range(O)
        ]
        nb_ah_sb = load_small(nb_ah[:], [128, MU], F32, "nb_ah_sb")
        b1_ah_sb = load_small(b1_ah[:], [128, MU], F32, "b1_ah_sb")
        nb_oh_sb = [
            load_small(nb_oh[o], [128, MU], F32, f"nb_oh_sb{o}") for o in range(O)
        ]
        b1_oh_sb = [
            load_small(b1_oh[o], [128, MU], F32, f"b1_oh_sb{o}") for o in range(O)
        ]
        b_pv_sb = load_small(b_pv[:], [PVP, 1 + O], F32, "b_pv_sb")
        ident_sb = load_small(ident[:], [128, 128], F16, "ident_sb")
        ones_sb = wp.tile([1, 128], F16)
        nc.vector.memset(ones_sb[:], 1.0)

        for m_it in range(niter * NMEGA):
            m = m_it % NMEGA
            bcol = m * NB

            # ---------- stage A: latents (transposed out, f-contraction) ----
            feat = fp.tile([128, KF, NB], F16, tag="feat")
            dma(
                out=feat[:],
                in_=xT.rearrange("(k p) b -> p k b", p=128)[
                    :, :, bcol : bcol + NB
                ],
            )

            lat1 = []  # 4 tiles [128, NB] fp16, chunk mc = u-dims [128mc, 128mc+128)
            for mc in range(MU):
                ps = psp.tile([128, NB], F32, tag="mm")
                for k in range(KF):
                    nc.tensor.matmul(
                        ps[:],
                        w_al_sb[:, k, 128 * mc : 128 * (mc + 1)],
                        feat[:, k, :],
                        start=(k == 0),
                        stop=(k == KF - 1),
                    )
                out = actp.tile([128, NB], F16, tag="lat1")
                _selu1_epilogue(
                    nc, pools, ps,
                    nb_al_sb[:, mc : mc + 1], b1_al_sb[:, mc : mc + 1], out[:],
                )
                lat1.append(out)

            opl1 = []  # [o][mc]
            for o in range(O):
                tiles = []
                for mc in range(MU):
                    ps = psp.tile([128, NB], F32, tag="mm")
                    for k in range(KF):
                        nc.tensor.matmul(
                            ps[:],
                            w_ol_sb[o][:, k, 128 * mc : 128 * (mc + 1)],
                            feat[:, k, :],
                            start=(k == 0),
                            stop=(k == KF - 1),
                        )
                    out = opp.tile([128, NB], F16, tag="opl1")
                    _selu1_epilogue(
                        nc, pools, ps,
                        nb_ol_sb[o][:, mc : mc + 1],
                        b1_ol_sb[o][:, mc : mc + 1],
                        out[:],
                    )
                    tiles.append(out)
                opl1.append(tiles)

            # ---------- stage B: q/k/v (natural layout) + attention ----------
            attn_T = [
                actp.tile([128, NB], F16, tag="attnT", name=f"attn_T{m}_{uc}")
                for uc in range(MU)
            ]
            for j in range(NB // 128):  # b-chunks of 128 within the mega-tile
                jcol = 128 * j

                def qkv_gemm(lat_tiles, w_sb, brow_sb, tag):
                    ps = psp.tile([128, NB], F32, tag="mm")
                    for k in range(KU):
                        nc.tensor.matmul(
                            ps[:],
                            lat_tiles[k][:, jcol : jcol + 128],
                            w_sb[:, k, :],
                            start=(k == 0),
                            stop=False,
                        )
                    # bias along the free dim via a K=1 ones matmul
                    nc.tensor.matmul(
                        ps[:], ones_sb[:], brow_sb[:], start=False, stop=True
                    )
                    sb = qkvp.tile([128, NB], F16, tag=tag)
                    nc.scalar.copy(sb[:], ps[:])
                    return sb

                q_sb = qkv_gemm(lat1, w_q_sb, bq_sb, "q")
                k_sbs = [qkv_gemm(opl1[o], w_k_sb, bk_sb, f"k{o}") for o in range(O)]
                v_sbs = [qkv_gemm(opl1[o], w_v_sb, bv_sb, f"v{o}") for o in range(O)]

                # scores[b,(o,h)] = sum_d q*k  -> [128, O*H] fp32
                sc = smp.tile([128, O * H], F32, tag="sc")
                for o in range(O):
                    prod = scr.tile([128, NB], F16, tag="prod", bufs=2)
                    nc.vector.tensor_mul(prod[:], q_sb[:], k_sbs[o][:])
                    nc.vector.tensor_reduce(
                        out=sc[:, H * o : H * (o + 1)],
                        in_=prod[:].rearrange("p (h d) -> p h d", h=H),
                        axis=mybir.AxisListType.X,
                        op=AO.add,
                    )
                # softmax over o (scores are tiny; skip the max-subtraction)
                e12 = smp.tile([128, O * H], F32, tag="e12")
                nc.scalar.activation(e12[:], sc[:], AF.Exp)
                s4 = smp.tile([128, H], F32, tag="s4")
                nc.vector.tensor_add(s4[:], e12[:, 0:H], e12[:, H : 2 * H])
                nc.vector.tensor_add(s4[:], s4[:], e12[:, 2 * H : 3 * H])
                r4 = smp.tile([128, H], F32, tag="r4")
                nc.vector.reciprocal(r4[:], s4[:])

                # opponent influences: infl[b,o] = mean_h w[b,h,o]
                infl = smp.tile([128, O], F32, tag="infl")
                wn = smp.tile([128, O * H], F32, tag="wn")
                for o in range(O):
                    nc.vector.scalar_tensor_tensor(
                        out=wn[:, H * o : H * (o + 1)],
                        in0=e12[:, H * o : H * (o + 1)],
                        scalar=1.0 / H,
                        in1=r4[:],
                        op0=AO.mult,
                        op1=AO.mult,
                    )
                nc.vector.tensor_reduce(
                    out=infl[:],
                    in_=wn[:].rearrange("p (o h) -> p o h", o=O),
                    axis=mybir.AxisListType.X,
                    op=AO.add,
                )
                dma(out=o_infl[bcol + jcol : bcol + jcol + 128, :], in_=infl[:])

                # attn (unnormalized): P2[:, o, (h d)] = e[b,(o,h)] * v_o[b,(h,d)]
                p2 = scr.tile([128, O * NB], F16, tag="p2", bufs=2)
                for o in range(O):
                    for h in range(H):
                        nc.vector.tensor_scalar(
                            out=p2[:, o * NB + HD * h : o * NB + HD * (h + 1)],
                            in0=v_sbs[o][:, HD * h : HD * (h + 1)],
                            scalar1=e12[:, H * o + h : H * o + h + 1],
                            scalar2=None,
                            op0=AO.mult,
                        )
                att_u = scr.tile([128, NB], F16, tag="att_u", bufs=2)
                nc.vector.tensor_add(att_u[:], p2[:, 0:NB], p2[:, NB : 2 * NB])
                nc.vector.tensor_add(att_u[:], att_u[:], p2[:, 2 * NB : 3 * NB])
                att_n = scr.tile([128, NB], F16, tag="att_n", bufs=2)
                for h in range(H):
                    nc.vector.tensor_scalar(
                        out=att_n[:, HD * h : HD * (h + 1)],
                        in0=att_u[:, HD * h : HD * (h + 1)],
                        scalar1=r4[:, h : h + 1],
                        scalar2=None,
                        op0=AO.mult,
                    )
                # transpose attn back to [u, b] chunks
                for uc in range(MU):
                    tp = psp.tile([128, 128], F16, tag="tp")
                    nc.tensor.transpose(
                        tp[:], att_n[:, 128 * uc : 128 * (uc + 1)], ident_sb[:]
                    )
                    nc.scalar.copy(attn_T[uc][:, jcol : jcol + 128], tp[:])

            # ---------- stage D: heads (transposed out) ----------
            head1 = []
            for mc in range(MU):
                ps = psp.tile([128, NB], F32, tag="mm")
                for k in range(2 * KU):
                    rhs = lat1[k][:] if k < KU else attn_T[k - KU][:]
                    nc.tensor.matmul(
                        ps[:],
                        w_ah_sb[:, k, 128 * mc : 128 * (mc + 1)],
                        rhs,
                        start=(k == 0),
                        stop=(k == 2 * KU - 1),
                    )
                out = actp.tile([128, NB], F16, tag="head1", bufs=4)
                _selu1_epilogue(
                    nc, pools, ps,
                    nb_ah_sb[:, mc : mc + 1], b1_ah_sb[:, mc : mc + 1], out[:],
                )
                head1.append(out)

            oh1 = []
            for o in range(O):
                tiles = []
                for mc in range(MU):
                    ps = psp.tile([128, NB], F32, tag="mm")
                    for k in range(KU):
                        nc.tensor.matmul(
                            ps[:],
                            w_oh_sb[o][:, k, 128 * mc : 128 * (mc + 1)],
                            opl1[o][k][:],
                            start=(k == 0),
                            stop=(k == KU - 1),
                        )
                    out = opp.tile([128, NB], F16, tag="oh1")
                    _selu1_epilogue(
                        nc, pools, ps,
                        nb_oh_sb[o][:, mc : mc + 1],
                        b1_oh_sb[o][:, mc : mc + 1],
                        out[:],
                    )
                    tiles.append(out)
                oh1.append(tiles)

            # ---------- stage E: policies + values (packed PV=7 rows) -------
            def pv_gemm(w_sb, head_tiles, bias_col, out_dram_slice):
                ps = psp.tile([PVP, NB], F32, tag="mm")
                for k in range(KU):
                    nc.tensor.matmul(
                        ps[:],
                        w_sb[:, k, :],
                        head_tiles[k][:],
                        start=(k == 0),
                        stop=(k == KU - 1),
                    )
                sb = smp.tile([PVP, NB], F32, tag="pv", bufs=2)
                nc.scalar.activation(
                    sb[:], ps[:], AF.Identity,
                    bias=b_pv_sb[:, bias_col : bias_col + 1], scale=1.0,
                )
                dma(out=out_dram_slice, in_=sb[0:PV, :])

            pv_gemm(w_pva_sb, head1, 0, o_agpv[:, bcol : bcol + NB])
            for o in range(O):
                pv_gemm(
                    w_pvo_sb[o], oh1[o], 1 + o, o_oppv[o][:, bcol : bcol + NB]
                )

    nc.finalize()
    return nc


def pack_inputs(features, W_al, b_al, W_in, b_in, W_out, b_out, W_ah, b_ah,
                W_ap, b_ap, W_av, b_av, W_ol, b_ol, W_oh, b_oh, W_op, b_op,
                W_ov, b_ov):
    """Host-side packing: transpose/cast weights, fold scale/shift corrections."""
    f32 = np.float32
    f16 = np.float16
    np32 = lambda a: np.asarray(a, dtype=f32)

    features = np32(features)
    W_al, b_al = np32(W_al), np32(b_al)
    W_in, b_in = np32(W_in), np32(b_in)
    W_out, b_out = np32(W_out), np32(b_out)
    W_ah, b_ah = np32(W_ah), np32(b_ah)
    W_ap, b_ap = np32(W_ap), np32(b_ap)
    W_av, b_av = np32(W_av), np32(b_av)
    W_ol, b_ol = np32(W_ol), np32(b_ol)
    W_oh, b_oh = np32(W_oh), np32(b_oh)
    W_op, b_op = np32(W_op), np32(b_op)
    W_ov, b_ov = np32(W_ov), np32(b_ov)

    def bias_pair(b_eff):
        # per-partition bias tiles [128, MU] for the transposed-layout epilogue
        nb = (-b_eff).reshape(MU, 128).T.copy()
        b1 = (b_eff + 1.0).reshape(MU, 128).T.copy()
        return nb.astype(f32), b1.astype(f32)

    sc = 1.0 / np.sqrt(HD)
    Wq, Wk, Wv = W_in[:U] * sc, W_in[U : 2 * U], W_in[2 * U :]
    bq, bk, bv = b_in[:U] * sc, b_in[U : 2 * U], b_in[2 * U :]
    # +1-shift corrections: consumers of shifted activations subtract W @ 1
    bq_eff = bq - Wq.sum(1)
    bk_eff = bk - Wk.sum(1)
    bv_eff = bv - Wv.sum(1)

    W_ah1 = W_ah[:, :U]
    W_ah2 = W_ah[:, U:] @ W_out  # fold out-proj into the agent-head GEMM
    b_ah_eff = b_ah + W_ah[:, U:] @ b_out - W_ah1.sum(1)
    b_oh_eff = b_oh - W_oh.sum(2)  # [O, U]

    nb_al, b1_al = bias_pair(b_al)
    nb_ah, b1_ah = bias_pair(b_ah_eff)
    nb_ol = np.stack([bias_pair(b_ol[o])[0] for o in range(O)])
    b1_ol = np.stack([bias_pair(b_ol[o])[1] for o in range(O)])
    nb_oh = np.stack([bias_pair(b_oh_eff[o])[0] for o in range(O)])
    b1_oh = np.stack([bias_pair(b_oh_eff[o])[1] for o in range(O)])

    w_pv_ag = np.zeros((PVP, U), f32)
    w_pv_ag[:PV] = np.concatenate([W_ap, W_av], axis=0)     # [7, U] used
    b_pv_ag = np.zeros((PVP,), f32)
    b_pv_ag[:PV] = np.concatenate([b_ap - W_ap.sum(1), b_av - W_av.sum(1)])
    w_pv_op = np.zeros((O, PVP, U), f32)
    w_pv_op[:, :PV] = np.concatenate([W_op, W_ov], axis=1)  # [O, 7, U] used
    b_pv_op = np.zeros((O, PVP), f32)
    b_pv_op[:, :PV] = np.concatenate(
        [b_op - W_op.sum(2), b_ov - W_ov.sum(2)], axis=1
    )
    b_pv = np.concatenate([b_pv_ag[:, None], b_pv_op.transpose(1, 0)], axis=1).astype(f32)

    shared = {
        "w_al_T": np.ascontiguousarray(W_al.T, dtype=f16),
        "w_ol_T": np.ascontiguousarray(W_ol.transpose(0, 2, 1), dtype=f16),
        "w_q_T": np.ascontiguousarray(Wq.T, dtype=f16),
        "w_k_T": np.ascontiguousarray(Wk.T, dtype=f16),
        "w_v_T": np.ascontiguousarray(Wv.T, dtype=f16),
        "bq_row": bq_eff.reshape(1, U).astype(f16),
        "bk_row": bk_eff.reshape(1, U).astype(f16),
        "bv_row": bv_eff.reshape(1, U).astype(f16),
        "w_ah_T": np.ascontiguousarray(
            np.concatenate([W_ah1.T, W_ah2.T], axis=0), dtype=f16
        ),
        "w_oh_T": np.ascontiguousarray(W_oh.transpose(0, 2, 1), dtype=f16),
        "w_pv_ag_T": np.ascontiguousarray(w_pv_ag.T, dtype=f16),
        "w_pv_op_T": np.ascontiguousarray(w_pv_op.transpose(0, 2, 1), dtype=f16),
        "nb_al": nb_al, "b1_al": b1_al,
        "nb_ol": nb_ol, "b1_ol": b1_ol,
        "nb_ah": nb_ah, "b1_ah": b1_ah,
        "nb_oh": nb_oh, "b1_oh": b1_oh,
        "b_pv": b_pv,
        "ident": np.eye(128, dtype=f16),
    }

    xT_full = np.ascontiguousarray(features.T, dtype=f16)  # [F, B]
    in_maps = []
    for c in range(NCORES):
        m = dict(shared)
        m["xT"] = np.ascontiguousarray(xT_full[:, c * BS : (c + 1) * BS])
        in_maps.append(m)
    return in_maps


def assemble_outputs(results):
    """results: per-core dicts -> full-size output tuple (all fp32)."""
    agp, agv, opp_, opv, infl = [], [], [], [], []
    for r in results:
        ag = r["out_ag_pv"]            # [7, BS]
        op = r["out_op_pv"]            # [O, 7, BS]
        agp.append(ag[:A].T)           # [BS, A]
        agv.append(ag[A : A + 1].T)    # [BS, 1]
        opp_.append(op[:, :A].transpose(2, 0, 1))      # [BS, O, A]
        opv.append(op[:, A : A + 1].transpose(2, 0, 1))  # [BS, O, 1]
        infl.append(r["out_infl"])     # [BS, O]
    cat = lambda xs: np.ascontiguousarray(np.concatenate(xs, axis=0), np.float32)
    return (cat(agp), cat(agv), cat(opp_), cat(opv), cat(infl))


_NC_CACHE = None


def get_nc():
    global _NC_CACHE
    if _NC_CACHE is None:
        _NC_CACHE = build_nc()
    return _NC_CACHE


def kernel(**inputs):
    from concourse.bass_utils import run_bass_kernel_spmd

    nc = get_nc()
    in_maps = pack_inputs(**inputs)
    res = run_bass_kernel_spmd(nc, in_maps, list(range(NCORES)))
    return assemble_outputs(res.results)
